# revision 1
# baseline (speedup 1.0000x reference)
"""Differentiable-FK forward kernel for Trainium2 (8 NeuronCores, data-parallel).

Problem: batch B=131072 of kinematic chains (63 bodies: world, free root,
61 hinges), 16 sites gathered from bodies. Output [B, 16, 3] site positions.

Strategy: pure data-parallel across 8 cores (16384 rows each). Per core the
batch is laid out as [128 partitions x 128 free]. The hinge chain is evaluated
sequentially (61 quaternion composes) on the Vector engine with the Scalar
engine supplying bulk sin/cos. Positions use a telescoped accumulation

    out_s(m) = wp1 + Rt(wq_1)K_2 + sum_{j=2}^{m-1} Rt(wq_j) G_j
             + Rt(wq_m)(sp_s - jp_m) + CONST_s

where Rt(q)v = R(q)v - v is the pure-quadratic part of the rotation and all
constant vectors (G_j, CONST_s) are host-precomputed from the tiny tree
tensors and baked into instruction immediates. The kernel is recompiled (and
disk-cached) per unique set of tree constants; qpos is the only streamed
input.
"""
import hashlib
import numpy as np

import concourse.bacc as bacc
import concourse.mybir as mybir
from concourse.tile import TileContext
from concourse.bass_utils import run_bass_kernel_spmd

F32 = mybir.dt.float32
MULT = mybir.AluOpType.mult
ADD = mybir.AluOpType.add
SUB = mybir.AluOpType.subtract

B_FULL = 131072
NCORES = 8
N = B_FULL // NCORES          # 16384 rows per core
P = 128                       # partitions
F = N // P                    # 128 free
NBODY = 63
NH = NBODY - 2                # 61
NQ = 7 + NH                   # 68
NSITES = 16
OUTW = NSITES * 3             # 48

_CACHE = {}


def _qmul_np(q1, q2):
    w1, x1, y1, z1 = [q1[..., i] for i in range(4)]
    w2, x2, y2, z2 = [q2[..., i] for i in range(4)]
    return np.stack([
        w1 * w2 - x1 * x2 - y1 * y2 - z1 * z2,
        w1 * x2 + x1 * w2 + y1 * z2 - z1 * y2,
        w1 * y2 - x1 * z2 + y1 * w2 + z1 * x2,
        w1 * z2 + x1 * y2 - y1 * x2 + z1 * w2,
    ], -1)


def _build(consts_key, body_pos, body_quat, hinge_axis, jnt_pos, site_pos,
           body_parent, site_body):
    # ---- host constant precompute ----
    parents = np.asarray(body_parent).astype(np.int64)
    sbody = np.asarray(site_body).astype(np.int64)
    assert np.array_equal(parents, np.maximum(np.arange(NBODY) - 1, 0)), \
        "kernel specialized for chain topology"

    A = np.asarray(body_quat[2:], np.float64)                      # [NH,4]
    Bq = _qmul_np(np.asarray(body_quat[2:], np.float64),
                  np.concatenate([np.zeros((NH, 1)), np.asarray(hinge_axis, np.float64)], -1))
    K = np.asarray(body_pos[2:], np.float64) + np.asarray(jnt_pos, np.float64)  # K_m, m=2..62
    jp = np.asarray(jnt_pos, np.float64)
    sp = np.asarray(site_pos, np.float64)

    G = np.zeros((NH, 3))
    for h in range(NH):
        G[h] = (K[h + 1] if h + 1 < NH else 0.0) - jp[h]
    # constant prefix C_m = K_2 + sum_{j=2}^{m-1} G_j
    Cpre = np.zeros((NBODY, 3))
    acc = K[0].copy()
    for m in range(2, NBODY):
        Cpre[m] = acc
        acc = acc + G[m - 2]

    site_by_body = {}
    for s, m in enumerate(sbody):
        site_by_body.setdefault(int(m), []).append(s)
    m_max = max(site_by_body.keys())

    # ---- bass program ----
    nc = bacc.Bacc("TRN2")
    qpos_d = nc.dram_tensor("qpos", [N, NQ], F32, kind="ExternalInput")
    out_d = nc.dram_tensor("sites", [N, OUTW], F32, kind="ExternalOutput")

    with TileContext(nc) as tc:
        with tc.tile_pool(name="main", bufs=1) as pool, \
             tc.tile_pool(name="scratch", bufs=2) as sp_pool:

            QP = pool.tile([P, F * NQ], F32)
            nc.sync.dma_start(QP[:], qpos_d[:].rearrange("(p f) k -> p (f k)", p=P))
            QPr = QP[:].rearrange("p (f k) -> p k f", k=NQ)   # [P, 68, F] strided view

            s_all = pool.tile([P, NH * F], F32)
            c_all = pool.tile([P, NH * F], F32)
            scr = pool.tile([P, NH * F], F32)
            ang = QPr[:, 7:NQ, :]                              # [P, 61, F]
            s3 = s_all[:].rearrange("p (h f) -> p h f", h=NH)
            c3 = c_all[:].rearrange("p (h f) -> p h f", h=NH)
            x3 = scr[:].rearrange("p (h f) -> p h f", h=NH)
            Sin, Square, Sqrt = (mybir.ActivationFunctionType.Sin,
                                 mybir.ActivationFunctionType.Square,
                                 mybir.ActivationFunctionType.Sqrt)
            # order matters: s_all doubles as the sin(th/4)^2 scratch first
            nc.scalar.activation(x3, ang, Sin, bias=0.0, scale=0.25)
            nc.scalar.activation(s3, x3, Square, bias=0.0, scale=1.0)
            # c = 1 - 2*sin^2(theta/4)
            nc.vector.tensor_scalar(c3, s3, -2.0, 1.0, MULT, ADD)
            nc.scalar.activation(s3, ang, Sin, bias=0.0, scale=0.5)

            def tile(tag):
                return sp_pool.tile([P, F], F32, tag=tag, name=tag)

            # ---- root ----
            rq = [QPr[:, 3 + i, :] for i in range(4)]
            n2 = tile("n2")
            nc.vector.tensor_tensor(n2, rq[0], rq[0], MULT)
            for i in range(1, 4):
                t = tile("rt")
                nc.vector.tensor_tensor(t, rq[i], rq[i], MULT)
                nc.vector.tensor_tensor(n2, n2, t, ADD)
            rn = sp_pool.tile([P, F], F32, tag="rn", name="rn")
            nc.scalar.activation(rn, n2, Sqrt, bias=0.0, scale=1.0)
            nc.vector.reciprocal(rn, rn)
            wq = [sp_pool.tile([P, F], F32, tag=f"wq{i}", name=f"wq{i}") for i in range(4)]
            for i in range(4):
                nc.vector.tensor_tensor(wq[i], rq[i], rn, MULT)

            Pacc = [pool.tile([P, F], F32, name=f"Pacc{i}") for i in range(3)]
            for i in range(3):
                nc.vector.tensor_copy(Pacc[i], QPr[:, i, :])

            OS = pool.tile([P, F * OUTW], F32)
            OSr = OS[:].rearrange("p (f k) -> p k f", k=OUTW)  # [P, 48, F]

            def emit_rot_sites_and_G(q, body, vecs):
                """q = [w,x,y,z] tiles. vecs = list of (v3, const3_or_None, out_idx).
                For each vec: if out_idx is None -> P += Rt(q)v  (position update)
                else OSr[:, out_idx..] = P + Rt(q)v + const (site emit, 3 comps).
                """
                w, u = q[0], q[1:4]
                for (v, cst, oidx) in vecs:
                    v2 = [2.0 * float(v[i]) for i in range(3)]
                    tx, ty, tz = tile("tx"), tile("ty"), tile("tz")
                    # t = cross(u, 2v)
                    for (to, ia, ib, ca, cb) in ((tx, 1, 2, v2[2], v2[1]),
                                                 (ty, 2, 0, v2[0], v2[2]),
                                                 (tz, 0, 1, v2[1], v2[0])):
                        m = tile("m")
                        nc.vector.tensor_scalar(m, u[ia], ca, None, MULT)
                        nc.vector.scalar_tensor_tensor(to, u[ib], -cb, m, MULT, ADD)
                    tv = (tx, ty, tz)
                    # r_i = w*t_i + (u x t)_i ; then P+= or site out
                    for ci, (ia, ib) in enumerate(((1, 2), (2, 0), (0, 1))):
                        a1 = tile("a1"); a2 = tile("a2"); a4 = tile("a4")
                        nc.vector.tensor_tensor(a1, w, tv[ci], MULT)
                        nc.vector.tensor_tensor(a2, u[ia], tv[ib], MULT)
                        nc.vector.tensor_tensor(a1, a1, a2, ADD)
                        nc.vector.tensor_tensor(a4, u[ib], tv[ia], MULT)
                        nc.vector.tensor_tensor(a1, a1, a4, SUB)
                        if oidx is None:
                            nc.vector.tensor_tensor(Pacc[ci], Pacc[ci], a1, ADD)
                        else:
                            nc.vector.scalar_tensor_tensor(
                                OSr[:, oidx + ci, :], a1, float(cst[ci]),
                                Pacc[ci], ADD, ADD)

            # sites on body 1 (root): out = wp1 + Rt(wq1) sp + sp, using Pacc==wp1
            for sid in site_by_body.get(1, []):
                emit_rot_sites_and_G(wq, 1, [(sp[sid], sp[sid], 3 * sid)])

            # P init: += Rt(wq1) K_2
            emit_rot_sites_and_G(wq, 1, [(K[0], None, None)])

            # ---- chain ----
            for j in range(2, m_max + 1):
                h = j - 2
                sh = s3[:, h, :]
                ch = c3[:, h, :]
                # lq = c*A + s*B
                lq = []
                for i in range(4):
                    ti = tile("lqt")
                    nc.vector.tensor_scalar(ti, sh, float(Bq[h, i]), None, MULT)
                    li = sp_pool.tile([P, F], F32, tag=f"lq{i}", name=f"lq{i}")
                    nc.vector.scalar_tensor_tensor(li, ch, float(A[h, i]), ti, MULT, ADD)
                    lq.append(li)
                # nq = wq x lq : 16 products
                pr = {}
                for a in range(4):
                    for b in range(4):
                        pab = sp_pool.tile([P, F], F32, tag=f"pr{a}{b}", name=f"pr{a}{b}")
                        nc.vector.tensor_tensor(pab, wq[a], lq[b], MULT)
                        pr[(a, b)] = pab
                nq = [sp_pool.tile([P, F], F32, tag=f"wq{i}", name=f"nq{i}") for i in range(4)]
                combos = [
                    (0, (0, 0), [((1, 1), SUB), ((2, 2), SUB), ((3, 3), SUB)]),
                    (1, (0, 1), [((1, 0), ADD), ((2, 3), ADD), ((3, 2), SUB)]),
                    (2, (0, 2), [((1, 3), SUB), ((2, 0), ADD), ((3, 1), ADD)]),
                    (3, (0, 3), [((1, 2), ADD), ((2, 1), SUB), ((3, 0), ADD)]),
                ]
                for (i, first, rest) in combos:
                    nc.vector.tensor_tensor(nq[i], pr[first], pr[rest[0][0]], rest[0][1])
                    for (key, op) in rest[1:]:
                        nc.vector.tensor_tensor(nq[i], nq[i], pr[key], op)
                wq = nq
                # sites on body j (before G update), then G update
                vecs = []
                for sid in site_by_body.get(j, []):
                    v = sp[sid] - jp[h]
                    vecs.append((v, Cpre[j] + v, 3 * sid))
                if j < m_max:
                    vecs.append((G[h], None, None))
                emit_rot_sites_and_G(wq, j, vecs)

            nc.sync.dma_start(out_d[:].rearrange("(p f) k -> p (f k)", p=P), OS[:])

    nc.compile()
    return nc


def _get_nc(inputs):
    key_src = b"".join(np.ascontiguousarray(np.asarray(inputs[k])).tobytes()
                       for k in ("body_pos", "body_quat", "hinge_axis", "jnt_pos",
                                 "site_pos", "body_parent", "site_body"))
    key = hashlib.sha256(key_src).hexdigest()
    if key not in _CACHE:
        _CACHE[key] = _build(key, inputs["body_pos"], inputs["body_quat"],
                             inputs["hinge_axis"], inputs["jnt_pos"],
                             inputs["site_pos"], inputs["body_parent"],
                             inputs["site_body"])
    return _CACHE[key]


def kernel(**inputs) -> np.ndarray:
    qpos = np.ascontiguousarray(np.asarray(inputs["qpos"], dtype=np.float32))
    assert qpos.shape == (B_FULL, NQ)
    nc = _get_nc(inputs)
    in_maps = [{"qpos": qpos[c * N:(c + 1) * N]} for c in range(NCORES)]
    res = run_bass_kernel_spmd(nc, in_maps, list(range(NCORES)))
    out = np.concatenate([res.results[c]["sites"] for c in range(NCORES)], axis=0)
    return out.reshape(B_FULL, NSITES, 3)


if __name__ == "__main__":
    rng = np.random.RandomState(0)
    import importlib.util
    spec = importlib.util.spec_from_file_location("reference", "/root/problem/reference.py")
    ref = importlib.util.module_from_spec(spec)
    spec.loader.exec_module(ref)
    inputs = {k: np.asarray(v) for k, v in ref.setup_inputs().items()}
    out = kernel(**inputs)
    print("out", out.shape, out.dtype)



# revision 9
# speedup vs baseline: 107.7354x; 107.7354x over previous
"""Differentiable-FK forward kernel for Trainium2 (8 NeuronCores, data-parallel).

Problem: batch B=131072 of kinematic chains (63 bodies: world, free root,
61 hinges), 16 sites gathered from bodies. Output [B, 16, 3] site positions.

Strategy: pure data-parallel across 8 cores (16384 rows each). Per core the
batch is laid out as [128 partitions x 128 free]. The hinge chain is evaluated
sequentially (61 quaternion composes). Positions use a telescoped accumulation

    out_s(m) = wp1 + Rt(wq_1)K_2 + sum_{j=2}^{m-1} Rt(wq_j) G_j
             + Rt(wq_m)(sp_s - jp_m) + CONST_s

where Rt(q)v = R(q)v - v is the pure-quadratic part of the rotation and all
constant vectors (G_j, CONST_s) are host-precomputed from the tiny tree
tensors and baked into instruction immediates. The kernel is recompiled (and
NEFF-cached) per unique set of tree constants; qpos is the only streamed
input.

Elementwise work is spread across the three elementwise-capable engines
(Vector/DVE, GpSimd/Pool, Scalar/Act) with a greedy cost-weighted balancer;
the Activation engine also supplies the bulk half-angle sin/cos.
"""
import hashlib
import numpy as np

import concourse.bacc as bacc
import concourse.mybir as mybir
from concourse.tile import TileContext
from concourse.bass_utils import run_bass_kernel_spmd

F32 = mybir.dt.float32
MULT = mybir.AluOpType.mult
ADD = mybir.AluOpType.add
SUB = mybir.AluOpType.subtract

B_FULL = 131072
NCORES = 8
N = B_FULL // NCORES          # 16384 rows per core
P = 128                       # partitions
F = N // P                    # 128 free
NBODY = 63
NH = NBODY - 2                # 61
NQ = 7 + NH                   # 68
NSITES = 16
OUTW = NSITES * 3             # 48

HALF_PI = float(np.pi / 2)

# per-op cost weights (ns) used by the greedy engine balancer.
# HW-calibrated on trn2 via hwcal.py (independent [128,128] f32 ops):
#   dve: tt 229, ts 197, stt 288; pool: tt 436, ts 1965, copy 586;
#   act(copy) 326. Pool is excluded from ts (catastrophically slow) and
#   neuronxcc rejects scalar_tensor_tensor on Pool.
COSTS = {
    "dve":  {"tt": 229.0, "ts": 197.0, "stt": 288.0, "cp": 197.0},
    "pool": {"tt": 436.0, "cp": 586.0},
    "act":  {"ts": 326.0, "cp": 326.0},
}

_CACHE = {}


def _qmul_np(q1, q2):
    w1, x1, y1, z1 = [q1[..., i] for i in range(4)]
    w2, x2, y2, z2 = [q2[..., i] for i in range(4)]
    return np.stack([
        w1 * w2 - x1 * x2 - y1 * y2 - z1 * z2,
        w1 * x2 + x1 * w2 + y1 * z2 - z1 * y2,
        w1 * y2 - x1 * z2 + y1 * w2 + z1 * x2,
        w1 * z2 + x1 * y2 - y1 * x2 + z1 * w2,
    ], -1)


class Bal:
    """Greedy per-op engine balancer across DVE / Pool / Act."""

    def __init__(self, nc):
        self.nc = nc
        self.load = {"dve": 0.0, "pool": 0.0, "act": 0.0}

    def _pick(self, optype, eligible):
        e = min(eligible, key=lambda e: self.load[e] + COSTS[e][optype])
        self.load[e] += COSTS[e][optype]
        return e

    def tt(self, out, a, b, op):
        e = self._pick("tt", ("dve", "pool"))
        eng = self.nc.vector if e == "dve" else self.nc.gpsimd
        eng.tensor_tensor(out, a, b, op)

    def stt(self, out, a, scal, b, op0, op1):
        # neuronxcc rejects TensorScalarPtr (scalar_tensor_tensor) on Pool
        e = self._pick("stt", ("dve",))
        self.nc.vector.scalar_tensor_tensor(out, a, scal, b, op0, op1)

    def ts_mult(self, out, a, scal):
        e = self._pick("ts", ("dve", "act"))
        if e == "act":
            self.nc.scalar.activation(out, a, mybir.ActivationFunctionType.Copy,
                                      bias=0.0, scale=float(scal))
        else:
            self.nc.vector.tensor_scalar(out, a, float(scal), None, MULT)

    def cp(self, out, a):
        e = self._pick("cp", ("dve", "act"))
        if e == "act":
            self.nc.scalar.activation(out, a, mybir.ActivationFunctionType.Copy,
                                      bias=0.0, scale=1.0)
        else:
            self.nc.vector.tensor_copy(out, a)


def _build(body_pos, body_quat, hinge_axis, jnt_pos, site_pos,
           body_parent, site_body, loop_iters=None):
    # ---- host constant precompute ----
    parents = np.asarray(body_parent).astype(np.int64)
    sbody = np.asarray(site_body).astype(np.int64)
    assert np.array_equal(parents, np.maximum(np.arange(NBODY) - 1, 0)), \
        "kernel specialized for chain topology"

    A = np.asarray(body_quat[2:], np.float64)                      # [NH,4]
    Bq = _qmul_np(np.asarray(body_quat[2:], np.float64),
                  np.concatenate([np.zeros((NH, 1)), np.asarray(hinge_axis, np.float64)], -1))
    K = np.asarray(body_pos[2:], np.float64) + np.asarray(jnt_pos, np.float64)
    jp = np.asarray(jnt_pos, np.float64)
    sp = np.asarray(site_pos, np.float64)

    G = np.zeros((NH, 3))
    for h in range(NH):
        G[h] = (K[h + 1] if h + 1 < NH else 0.0) - jp[h]
    Cpre = np.zeros((NBODY, 3))
    acc = K[0].copy()
    for m in range(2, NBODY):
        Cpre[m] = acc
        acc = acc + G[m - 2]

    site_by_body = {}
    for s, m in enumerate(sbody):
        site_by_body.setdefault(int(m), []).append(s)
    m_max = max(site_by_body.keys())

    # ---- bass program ----
    nc = bacc.Bacc("TRN2")
    qpos_d = nc.dram_tensor("qpos", [N, NQ], F32, kind="ExternalInput")
    out_d = nc.dram_tensor("sites", [N, OUTW], F32, kind="ExternalOutput")

    Sin, Sqrt = (mybir.ActivationFunctionType.Sin,
                 mybir.ActivationFunctionType.Sqrt)

    with TileContext(nc) as tc:
        with tc.tile_pool(name="main", bufs=1) as pool, \
             tc.tile_pool(name="scratch", bufs=2) as sp_pool:

            bal = Bal(nc)

            # persistent allocations, shared across benchmark-loop iterations
            QP = pool.tile([P, F * NQ], F32)
            QPr = QP[:].rearrange("p (f k) -> p k f", k=NQ)       # [P, 68, F]
            s_all = pool.tile([P, NH * F], F32)
            c_all = pool.tile([P, NH * F], F32)
            s3 = s_all[:].rearrange("p (h f) -> p h f", h=NH)
            c3 = c_all[:].rearrange("p (h f) -> p h f", h=NH)
            halfpi = pool.tile([P, 1], F32, name="halfpi")
            nc.vector.memset(halfpi[:], HALF_PI)
            PaccT = [pool.tile([P, F], F32, name=f"Pacc{i}") for i in range(3)]
            OS = pool.tile([P, F * OUTW], F32)
            OSr = OS[:].rearrange("p (f k) -> p k f", k=OUTW)     # [P, 48, F]

            for _it in range(loop_iters or 1):
                nc.sync.dma_start(QP[:], qpos_d[:].rearrange("(p f) k -> p (f k)", p=P))
                ang = QPr[:, 7:NQ, :]                              # [P, 61, F]
                # s = sin(theta/2), c = sin(theta/2 + pi/2) = cos(theta/2)
                nc.scalar.activation(s3, ang, Sin, bias=0.0, scale=0.5)
                nc.scalar.activation(c3, ang, Sin, bias=halfpi[:], scale=0.5)
                bal.load["act"] += 2 * 6700.0  # bulk trig occupancy

                def tile(tag):
                    return sp_pool.tile([P, F], F32, tag=tag, name=tag)

                # ---- root: wq = normalize(qpos[:, 3:7]) ----
                rq = [QPr[:, 3 + i, :] for i in range(4)]
                n2 = tile("n2")
                bal.tt(n2, rq[0], rq[0], MULT)
                for i in range(1, 4):
                    t = tile(f"rt{i}")
                    bal.tt(t, rq[i], rq[i], MULT)
                    bal.tt(n2, n2, t, ADD)
                rn = sp_pool.tile([P, F], F32, tag="rn", name="rn")
                nc.scalar.activation(rn, n2, Sqrt, bias=0.0, scale=1.0)
                nc.vector.reciprocal(rn, rn)
                wq = [sp_pool.tile([P, F], F32, tag=f"wq{i}", name=f"wq{i}") for i in range(4)]
                for i in range(4):
                    bal.tt(wq[i], rq[i], rn, MULT)

                Pacc = PaccT
                for i in range(3):
                    bal.cp(Pacc[i], QPr[:, i, :])

                def emit_rot(q, vecs):
                    """q = [w,x,y,z] tiles. vecs = list of (v3, const3_or_None, out_idx).
                    out_idx None -> Pacc += Rt(q)v ; else OSr[:, oidx..] = Pacc + Rt(q)v + const.
                    """
                    w, u = q[0], q[1:4]
                    for (v, cst, oidx) in vecs:
                        v2 = [2.0 * float(v[i]) for i in range(3)]
                        tx, ty, tz = tile("tx"), tile("ty"), tile("tz")
                        # t = cross(u, 2v)
                        for (to, ia, ib, ca, cb) in ((tx, 1, 2, v2[2], v2[1]),
                                                     (ty, 2, 0, v2[0], v2[2]),
                                                     (tz, 0, 1, v2[1], v2[0])):
                            m = tile("m")
                            bal.ts_mult(m, u[ia], ca)
                            bal.stt(to, u[ib], -cb, m, MULT, ADD)
                        tv = (tx, ty, tz)
                        # r_i = w*t_i + (u x t)_i
                        for ci, (ia, ib) in enumerate(((1, 2), (2, 0), (0, 1))):
                            a1 = tile("a1"); a2 = tile("a2"); a4 = tile("a4")
                            bal.tt(a1, w, tv[ci], MULT)
                            bal.tt(a2, u[ia], tv[ib], MULT)
                            bal.tt(a1, a1, a2, ADD)
                            bal.tt(a4, u[ib], tv[ia], MULT)
                            bal.tt(a1, a1, a4, SUB)
                            if oidx is None:
                                bal.tt(Pacc[ci], Pacc[ci], a1, ADD)
                            else:
                                bal.stt(OSr[:, oidx + ci, :], a1, float(cst[ci]),
                                        Pacc[ci], ADD, ADD)

                # sites on body 1 (root)
                for sid in site_by_body.get(1, []):
                    emit_rot(wq, [(sp[sid], sp[sid], 3 * sid)])

                # P init: += Rt(wq1) K_2
                emit_rot(wq, [(K[0], None, None)])

                # ---- chain ----
                for j in range(2, m_max + 1):
                    h = j - 2
                    sh = s3[:, h, :]
                    ch = c3[:, h, :]
                    # lq = c*A + s*B
                    lq = []
                    for i in range(4):
                        ti = tile(f"lqt{i}")
                        bal.ts_mult(ti, sh, float(Bq[h, i]))
                        li = sp_pool.tile([P, F], F32, tag=f"lq{i}", name=f"lq{i}")
                        bal.stt(li, ch, float(A[h, i]), ti, MULT, ADD)
                        lq.append(li)
                    # nq = wq x lq : 16 products
                    pr = {}
                    for a in range(4):
                        for b in range(4):
                            pab = sp_pool.tile([P, F], F32, tag=f"pr{a}{b}", name=f"pr{a}{b}")
                            bal.tt(pab, wq[a], lq[b], MULT)
                            pr[(a, b)] = pab
                    nq = [sp_pool.tile([P, F], F32, tag=f"wq{i}", name=f"nq{i}") for i in range(4)]
                    combos = [
                        (0, (0, 0), [((1, 1), SUB), ((2, 2), SUB), ((3, 3), SUB)]),
                        (1, (0, 1), [((1, 0), ADD), ((2, 3), ADD), ((3, 2), SUB)]),
                        (2, (0, 2), [((1, 3), SUB), ((2, 0), ADD), ((3, 1), ADD)]),
                        (3, (0, 3), [((1, 2), ADD), ((2, 1), SUB), ((3, 0), ADD)]),
                    ]
                    for (i, first, rest) in combos:
                        bal.tt(nq[i], pr[first], pr[rest[0][0]], rest[0][1])
                        for (key, op) in rest[1:]:
                            bal.tt(nq[i], nq[i], pr[key], op)
                    wq = nq
                    # sites on body j, then G update
                    vecs = []
                    for sid in site_by_body.get(j, []):
                        v = sp[sid] - jp[h]
                        vecs.append((v, Cpre[j] + v, 3 * sid))
                    if j < m_max:
                        vecs.append((G[h], None, None))
                    emit_rot(wq, vecs)

                nc.sync.dma_start(out_d[:].rearrange("(p f) k -> p (f k)", p=P), OS[:])

    nc.compile()
    return nc


def _get_nc(inputs, loop_iters=None):
    key_src = b"".join(np.ascontiguousarray(np.asarray(inputs[k])).tobytes()
                       for k in ("body_pos", "body_quat", "hinge_axis", "jnt_pos",
                                 "site_pos", "body_parent", "site_body"))
    key = (hashlib.sha256(key_src).hexdigest(), loop_iters)
    if key not in _CACHE:
        _CACHE[key] = _build(inputs["body_pos"], inputs["body_quat"],
                             inputs["hinge_axis"], inputs["jnt_pos"],
                             inputs["site_pos"], inputs["body_parent"],
                             inputs["site_body"], loop_iters=loop_iters)
    return _CACHE[key]


def kernel(**inputs) -> np.ndarray:
    qpos = np.ascontiguousarray(np.asarray(inputs["qpos"], dtype=np.float32))
    assert qpos.shape == (B_FULL, NQ)
    nc = _get_nc(inputs)
    in_maps = [{"qpos": qpos[c * N:(c + 1) * N]} for c in range(NCORES)]
    res = run_bass_kernel_spmd(nc, in_maps, list(range(NCORES)))
    out = np.concatenate([res.results[c]["sites"] for c in range(NCORES)], axis=0)
    return out.reshape(B_FULL, NSITES, 3)


if __name__ == "__main__":
    import importlib.util
    spec = importlib.util.spec_from_file_location("reference", "/root/problem/reference.py")
    ref = importlib.util.module_from_spec(spec)
    spec.loader.exec_module(ref)
    inputs = {k: np.asarray(v) for k, v in ref.setup_inputs().items()}
    out = kernel(**inputs)
    print("out", out.shape, out.dtype)
